# revision 35
# baseline (speedup 1.0000x reference)
"""Trainium2 Bass kernel for causal multi-head attention (prefill), v2.

Problem: x[2,2048,768], 12 heads x 64 dim, causal softmax(QK^T/8)V + out-proj.

Sharding (8 cores, no collectives): core c handles batch c//4 and head group
c%4 (3 heads).  Host sums the 4 partial outputs per batch and transposes.

v2 changes vs the f32r baseline (292us):
 - fp16 everywhere on the matmul path (fp32r runs the PE in FP32_HIGH mode:
   no FWL weight loads, heavy HAM clock throttling -> ~1.2GHz effective).
   fp16 streams 1 col/cycle at 2.4GHz warm and halves DMA/LDWEIGHTS.
 - scores matmuls are K=64: pairs of them run CONCURRENTLY on disjoint
   PE row-groups (lhsT base partition 0 vs 64 -> auto tile_position).
   q/k are stored per head as [q;qdup]/[kdup;k] so even/odd kv tiles use
   disjoint row groups -> ~2x effective score throughput.
 - out-proj K=64 (wo2) matmuls likewise paired via duplicated wo2/ctx2.
 - reciprocal_approx_fast (PSUM src) instead of vector.reciprocal
   (40us -> ~7us), psum ctx pool double-buffered across windows.
 - emission order interleaves projections into attention window J=0 and
   out-proj(J) into attention(J+1) so the PE never sits behind the
   ACT-engine exp stream (strict per-engine FIFO).
 - fp16 output partials; host accumulates in fp32.
"""

import numpy as np

import concourse.bass as bass
import concourse.tile as tile
from concourse import bacc, mybir
from concourse.bass_utils import run_bass_kernel_spmd

F32 = mybir.dt.float32
F32R = mybir.dt.float32r
F16 = mybir.dt.float16

DEBUG = False

B, S, D = 2, 2048, 768
H, DH = 12, 64
HPC = 3                 # heads per core
NCORES = 8
KT = D // 128           # 6 contraction tiles for projections
NSQ = S // 512          # 4 col blocks of 512
NKV = S // 128          # 16 kv tiles of 128
WJ = 1024               # attention window width
NJ = S // WJ            # 2 windows


def build():
    nc = bacc.Bacc("TRN2", target_bir_lowering=False, debug=False)

    xT = nc.dram_tensor("xT", [D, S], F16, kind="ExternalInput")
    # per head h: cols [64h*2 .. ] = [Wq_h^T (64) | Wk_h^T (64)]
    wqk = nc.dram_tensor("wqk", [D, 384], F16, kind="ExternalInput")
    wv = nc.dram_tensor("wv", [D, 256], F16, kind="ExternalInput")  # 192+64 pad
    wo01 = nc.dram_tensor("wo01", [128, D], F16, kind="ExternalInput")
    wo2 = nc.dram_tensor("wo2", [64, D], F16, kind="ExternalInput")
    tri = nc.dram_tensor("tri", [128, 128], F16, kind="ExternalInput")
    negI = nc.dram_tensor("negI", [128, 128], F16, kind="ExternalInput")
    trilS = nc.dram_tensor("trilS", [128, 128], F16, kind="ExternalInput")
    onesd = nc.dram_tensor("onesd", [1, 64], F32, kind="ExternalInput")
    outT = nc.dram_tensor("outT", [D, S], F16, kind="ExternalOutput")
    if DEBUG:
        dbg_qkd0 = nc.dram_tensor("dbg_qkd0", [128, 2 * S], F16,
                                  kind="ExternalOutput")
        dbg_vaug = nc.dram_tensor("dbg_vaug", [128, NKV * 195], F16,
                                  kind="ExternalOutput")
        dbg_sp = nc.dram_tensor("dbg_sp", [256, WJ], F32, kind="ExternalOutput")
        dbg_esb = nc.dram_tensor("dbg_esb", [256, WJ], F16,
                                 kind="ExternalOutput")
        dbg_ctx = nc.dram_tensor("dbg_ctx", [65, WJ], F32,
                                 kind="ExternalOutput")
        dbg_inv = nc.dram_tensor("dbg_inv", [1, WJ], F32, kind="ExternalOutput")
        dbg_bsb = nc.dram_tensor("dbg_bsb", [64, WJ], F32,
                                 kind="ExternalOutput")

    with tile.TileContext(nc) as tc, \
         nc.allow_low_precision(reason="fp16 matmul path, fp32 psum accum"):
        with tc.tile_pool(name="sb", bufs=1) as sb, \
             tc.tile_pool(name="sbe", bufs=6) as sbe, \
             tc.tile_pool(name="sbo", bufs=2) as sbo, \
             tc.tile_pool(name="ps", bufs=2, space="PSUM") as ps, \
             tc.tile_pool(name="psc", bufs=1, space="PSUM") as psc:

            # ---- phase 0: load weights + x, first-needed first ----
            xsb = sb.tile([128, KT, S], F16, tag="xsb")
            x_r = xT[:, :].rearrange("(k p) n -> p k n", p=128)
            wqk_sb = sb.tile([128, KT, 384], F16, tag="wqk")
            wv_sb = sb.tile([128, KT, 256], F16, tag="wv")
            wo01_sb = sb.tile([128, D], F16, tag="wo01")
            wo2d_sb = sb.tile([128, D], F16, tag="wo2d")
            tri_sb = sb.tile([128, 128], F16, tag="tri")
            negI_sb = sb.tile([128, 128], F16, tag="negI")
            tril_sb = sb.tile([128, 128], F16, tag="trilS")
            ones_sb = sb.tile([1, 64], F32, tag="ones")

            nc.sync.dma_start(wqk_sb, wqk[:, :].rearrange("(k p) m -> p k m", p=128))
            nc.sync.dma_start(tri_sb, tri[:, :])
            nc.sync.dma_start(negI_sb, negI[:, :])
            nc.sync.dma_start(tril_sb, trilS[:, :])
            nc.sync.dma_start(ones_sb, onesd[:, :])
            for k in range(KT):
                nc.sync.dma_start(xsb[:, k, 0:1024], x_r[:, k, 0:1024])
            nc.sync.dma_start(wv_sb, wv[:, :].rearrange("(k p) m -> p k m", p=128))
            for k in range(KT):
                nc.sync.dma_start(xsb[:, k, 1024:2048], x_r[:, k, 1024:2048])
            nc.sync.dma_start(wo01_sb, wo01[:, :])
            nc.sync.dma_start(wo2d_sb[0:64, :], wo2[:, :])
            # duplicate wo2 rows into partitions 64:128 for paired out-proj
            nc.sync.dma_start(wo2d_sb[64:128, :], wo2[:, :])

            # ---- q/k storage: per head qkd[h] [128, 2, S] fp16
            #   [:, 0, :]: rows 0:64 = q_h, rows 64:128 = k_h
            #   [:, 1, :]: rows 0:64 = k_h dup, rows 64:128 = q_h dup
            # scores even kv tile: lhsT=k_dup@base0,  rhs=q@base0
            # scores odd  kv tile: lhsT=k@base64,     rhs=q_dup@base64
            qkd = [sb.tile([128, 2, S], F16, tag=f"qkd{h}", name=f"qkd{h}")
                   for h in range(HPC)]
            vaug = sb.tile([128, NKV, 195], F16, tag="vaug")
            ctxT01 = sb.tile([128, S], F16, tag="ctxT01")
            ctxT2d = sb.tile([128, S], F16, tag="ctxT2d")

            def proj_qk(nt):
                c0, c1 = nt * 512, (nt + 1) * 512
                for h in range(HPC):
                    pp = ps.tile([128, 512], F32, tag="pp", name="pp")
                    for k in range(KT):
                        nc.tensor.matmul(
                            pp, wqk_sb[:, k, h * 128:(h + 1) * 128],
                            xsb[:, k, c0:c1],
                            start=(k == 0), stop=(k == KT - 1))
                    nc.vector.tensor_copy(qkd[h][:, 0, c0:c1], pp)
                    # dup: q -> rows 64:128 of slot1; k -> rows 0:64 of slot1
                    nc.sync.dma_start(qkd[h][64:128, 1, c0:c1],
                                      qkd[h][0:64, 0, c0:c1])
                    nc.sync.dma_start(qkd[h][0:64, 1, c0:c1],
                                      qkd[h][64:128, 0, c0:c1])

            def proj_v(i):
                pp = ps.tile([128, 256], F32, tag="pp", name="pp")
                for k in range(KT):
                    nc.tensor.matmul(
                        pp, xsb[:, k, i * 128:(i + 1) * 128], wv_sb[:, k, :],
                        start=(k == 0), stop=(k == KT - 1))
                nc.vector.tensor_copy(
                    vaug[:, i, :].rearrange("p (h c) -> p h c", c=65)[:, :, 0:64],
                    pp[:, 0:192].rearrange("p (h c) -> p h c", c=64))

            def v_ones():
                for h in range(HPC):
                    nc.scalar.activation(
                        vaug[:, :, 65 * h + 64:65 * h + 65],
                        tri_sb[:, h * NKV:(h + 1) * NKV].rearrange(
                            "p (t c) -> p t c", c=1),
                        mybir.ActivationFunctionType.Copy, bias=1.0, scale=0.0)

            def kslc(h, i):
                half = i % 2
                if half == 0:   # k dup at base 0
                    return qkd[h][0:64, 1, i * 128:(i + 1) * 128]
                return qkd[h][64:128, 0, i * 128:(i + 1) * 128]

            def qslc(h, i, c0, c1):
                if i % 2 == 0:  # q at base 0
                    return qkd[h][0:64, 0, c0:c1]
                return qkd[h][64:128, 1, c0:c1]

            def attn_head(J, h, fillers, pre=None):
                """Attention for window J, head h, software-pipelined: ctx
                matmuls trail the score/exp stream by one kv-tile pair so the
                PE runs the previous pair's ctx while ACT computes the current
                exps.  `pre` (the previous head's normalize chain) is emitted
                after the first score pair.  Returns this head's normalize
                closure for the next head to emit."""
                ctx_ps = psc.tile([65, WJ], F32, tag="ctx", name="ctx_ps")
                imax = 8 * J + 7
                pending = []

                def emit_ctx(ent):
                    i, col0, nb0, esb = ent
                    for nb in range(nb0, 2):
                        s0 = max(nb * 512, col0)
                        nc.tensor.matmul(
                            ctx_ps[:, s0:(nb + 1) * 512],
                            vaug[:, i, 65 * h:65 * h + 65],
                            esb[:, s0:(nb + 1) * 512],
                            start=(i == 0), stop=(i == 8 * J + 4 * nb + 3))

                for ip in range(0, imax + 1, 2):
                    sps = []
                    for i in (ip, ip + 1):
                        d = 128 * i - WJ * J
                        col0 = max(0, d)
                        nb0 = max(0, d // 512)
                        spsum = ps.tile([128, WJ], F32, tag="sp", name="spsum")
                        for nb in range(nb0, 2):
                            s0 = max(nb * 512, col0)
                            nc.tensor.matmul(
                                spsum[:, s0:(nb + 1) * 512],
                                kslc(h, i),
                                qslc(h, i, WJ * J + s0, WJ * J + (nb + 1) * 512),
                                start=True, stop=not (d >= 0 and nb == nb0))
                        if d >= 0:
                            # additive causal mask on the diagonal block:
                            # spsum[:, d:d+128] += (-60000*I)^T @ strict_tril
                            nc.tensor.matmul(
                                spsum[:, d:d + 128], negI_sb, tril_sb,
                                start=False, stop=True)
                        sps.append((i, d, col0, nb0, spsum))
                    for i, d, col0, nb0, spsum in sps:
                        esb = sbe.tile([128, WJ], F16, tag="exp", name="esb")
                        if DEBUG and J == 0 and h == 0 and i < 2:
                            spc = sbo.tile([128, WJ], F32, tag="spc", name="spc")
                            nc.vector.tensor_copy(spc, spsum)
                            nc.sync.dma_start(dbg_sp[i * 128:(i + 1) * 128, :],
                                              spc)
                        nc.scalar.activation(
                            esb[:, col0:WJ], spsum[:, col0:WJ],
                            mybir.ActivationFunctionType.Exp, scale=0.125)
                        if DEBUG and J == 0 and h == 0 and i < 2:
                            nc.sync.dma_start(dbg_esb[i * 128:(i + 1) * 128, :],
                                              esb)
                        pending.append((i, col0, nb0, esb))
                    if ip == 2 and pre is not None:
                        pre()
                    while len(pending) > 4:
                        emit_ctx(pending.pop(0))
                    if fillers and ip >= 2:
                        fillers.pop(0)()
                while pending:
                    emit_ctx(pending.pop(0))

                def norm():
                    if DEBUG and J == 0 and h == 0:
                        ctxc = sbo.tile([65, WJ], F32, tag="ctxc", name="ctxc")
                        nc.vector.tensor_copy(ctxc, ctx_ps)
                        nc.sync.dma_start(dbg_ctx[:, :], ctxc)
                    # custom-DVE recip mishandles base-partition-64 PSUM
                    # sources; stage the denominator row to partition 0 first.
                    den_sb = sbo.tile([1, WJ], F32, tag="den", name="den_sb")
                    nc.vector.tensor_copy(den_sb, ctx_ps[64:65, :])
                    inv = sbo.tile([1, WJ], F32, tag="inv", name="inv")
                    nc.vector.reciprocal_approx_fast(out=inv, in_=den_sb)
                    if DEBUG and J == 0 and h == 0:
                        nc.sync.dma_start(dbg_inv[:, :], inv)
                    bsb = sbo.tile([64, WJ], F32, tag="bsb", name="bsb")
                    for nb in range(2):
                        bps = ps.tile([64, 512], F32, tag="pp", name="bps")
                        nc.tensor.matmul(bps, ones_sb,
                                         inv[:, nb * 512:(nb + 1) * 512],
                                         start=True, stop=True)
                        nc.vector.tensor_copy(
                            bsb[:, nb * 512:(nb + 1) * 512], bps)
                    if DEBUG and J == 0 and h == 0:
                        nc.sync.dma_start(dbg_bsb[:, :], bsb)
                    if h == 0:
                        dst = ctxT01[0:64, WJ * J:WJ * (J + 1)]
                    elif h == 1:
                        dst = sbo.tile([64, WJ], F16, tag="h1tmp", name="h1tmp")
                    else:
                        dst = ctxT2d[0:64, WJ * J:WJ * (J + 1)]
                    nc.vector.tensor_mul(dst, ctx_ps[0:64, :], bsb)
                    if h == 1:
                        nc.sync.dma_start(
                            ctxT01[64:128, WJ * J:WJ * (J + 1)], dst)
                    elif h == 2:
                        nc.sync.dma_start(
                            ctxT2d[64:128, WJ * J:WJ * (J + 1)],
                            ctxT2d[0:64, WJ * J:WJ * (J + 1)])
                return norm

            def outproj_block(j, m0):
                """out rows [m0*128:(m0+2)*128], cols [512j:512j+512].
                Two wo01 (K=128) matmuls then a concurrent pair of wo2
                (K=64) matmuls on disjoint row groups."""
                c0, c1 = j * 512, (j + 1) * 512
                opsA = ps.tile([128, 512], F32, tag="pp", name="opsA")
                opsB = ps.tile([128, 512], F32, tag="pp", name="opsB")
                nc.tensor.matmul(opsA, wo01_sb[:, m0 * 128:(m0 + 1) * 128],
                                 ctxT01[:, c0:c1], start=True, stop=False)
                nc.tensor.matmul(opsB, wo01_sb[:, (m0 + 1) * 128:(m0 + 2) * 128],
                                 ctxT01[:, c0:c1], start=True, stop=False)
                nc.tensor.matmul(opsA, wo2d_sb[0:64, m0 * 128:(m0 + 1) * 128],
                                 ctxT2d[0:64, c0:c1], start=False, stop=True)
                nc.tensor.matmul(opsB, wo2d_sb[64:128, (m0 + 1) * 128:(m0 + 2) * 128],
                                 ctxT2d[64:128, c0:c1], start=False, stop=True)
                for mt, ops in ((m0, opsA), (m0 + 1, opsB)):
                    osb = sbo.tile([128, 512], F16, tag="osb", name="osb")
                    nc.vector.tensor_copy(osb, ops)
                    nc.sync.dma_start(
                        outT[mt * 128:(mt + 1) * 128, c0:c1], osb)

            # ---- emission schedule ----
            # minimum prologue: q/k for window 0 and the first two v tiles;
            # everything else streams in through fillers between i-pairs.
            v_ones()
            proj_qk(0)
            proj_qk(1)
            proj_v(0)
            proj_v(1)

            def vv(a, b):
                def f():
                    proj_v(a)
                    proj_v(b)
                return f

            fill0 = [vv(2, 3), vv(4, 5), vv(6, 7),
                     lambda: proj_qk(2), lambda: proj_qk(3),
                     vv(8, 9), vv(10, 11), vv(12, 13), vv(14, 15)]
            pend = None
            for h in range(HPC):
                pend = attn_head(0, h, fill0, pre=pend)
            while fill0:
                fill0.pop(0)()

            fill1 = [lambda j=j, m0=m0: outproj_block(j, m0)
                     for j in (0, 1) for m0 in (0, 2, 4)]
            for h in range(HPC):
                pend = attn_head(1, h, fill1, pre=pend)
            pend()
            while fill1:
                fill1.pop(0)()
            for j in (2, 3):
                for m0 in (0, 2, 4):
                    outproj_block(j, m0)

            if DEBUG:
                nc.sync.dma_start(
                    dbg_qkd0[:, :], qkd[0].rearrange("p a n -> p (a n)"))
                nc.sync.dma_start(
                    dbg_vaug[:, :], vaug.rearrange("p a n -> p (a n)"))

    nc.compile()
    return nc


def shard_inputs(x, Wq, Wk, Wv, Wo):
    x = np.asarray(x, np.float32)
    Wq = np.asarray(Wq, np.float32)
    Wk = np.asarray(Wk, np.float32)
    Wv = np.asarray(Wv, np.float32)
    Wo = np.asarray(Wo, np.float32)
    tri = np.triu(np.ones((128, 128), np.float16))
    negI_a = (-60000.0 * np.eye(128)).astype(np.float16)
    trilS_a = np.tril(np.ones((128, 128), np.float32), -1).astype(np.float16)
    ones = np.ones((1, 64), np.float32)
    in_maps = []
    for c in range(NCORES):
        b, g = c // 4, c % 4
        rs = slice(192 * g, 192 * g + 192)
        qk_cols = []
        for h in range(HPC):
            hr = slice(192 * g + 64 * h, 192 * g + 64 * h + 64)
            qk_cols.append(Wq[hr].T)
            qk_cols.append(Wk[hr].T)
        wqk_t = np.concatenate(qk_cols, axis=1).astype(np.float16)
        wv_t = np.concatenate(
            [Wv[rs].T, np.zeros((D, 64), np.float32)], axis=1
        ).astype(np.float16)
        wo_t = np.ascontiguousarray(Wo[:, rs].T)
        in_maps.append({
            "xT": np.ascontiguousarray(x[b].T).astype(np.float16),
            "wqk": np.ascontiguousarray(wqk_t),
            "wv": np.ascontiguousarray(wv_t),
            "wo01": np.ascontiguousarray(wo_t[0:128]).astype(np.float16),
            "wo2": np.ascontiguousarray(wo_t[128:192]).astype(np.float16),
            "tri": tri,
            "negI": negI_a,
            "trilS": trilS_a,
            "onesd": ones,
        })
    return in_maps


def assemble(results, bo):
    out = np.zeros((B, S, D), np.float32)
    for c in range(NCORES):
        out[c // 4] += results[c]["outT"].T.astype(np.float32)
    return out + np.asarray(bo, np.float32)[None, None, :]


_NC = None


def kernel(x, Wq, Wk, Wv, Wo, bo, **run_kwargs):
    global _NC
    if _NC is None:
        _NC = build()
    in_maps = shard_inputs(x, Wq, Wk, Wv, Wo)
    res = run_bass_kernel_spmd(_NC, in_maps, core_ids=list(range(NCORES)),
                               **run_kwargs)
    out = assemble(res.results, bo)
    kernel.last_results = res
    return out
